# revision 1
# baseline (speedup 1.0000x reference)
"""Trainium2 Bass kernel for the AllGroupsExpertRunner MoE problem.

Math (dense-masked reference):
    x = tokens.reshape(M, D)                                # M = B*N = 8192
    out = sum_e w[:, e] * (gelu(x @ Wg[e]) * (x @ Wv[e])) @ Wo[e] * scales[e]
    where w = where(dispatch > 0, combine, 0)

Tokens with w[:, e] == 0 contribute nothing for expert e, so each expert only
needs its assigned tokens. Sharding: expert-parallel — core e runs expert e on
the tokens routed to it (gathered on host, padded to a common length NT), and
the host scatter-adds the 8 partial outputs. With top-2 routing that is ~2100
of 8192 tokens per expert (~4x less work than dense); with dense routing
weights it degrades gracefully to all tokens.

Per-core kernel: all matmuls run on the PE array in float32r (fp32 data at
full 1 cycle/row rate; ~2.8e-4 rel err measured on HW). x is passed
pre-transposed (D, NT) so no on-device transposes are needed:
  stage A (per token chunk of <=512, per 128-wide H block):
      g^T = Wg_blk^T @ xT-chunk  (PSUM, K=D via 4 accumulating matmuls)
      v^T = Wv_blk^T @ xT-chunk
      hT_blk = gelu(g^T) * v^T   (ACT + DVE)
  stage B (per 128-token sub-chunk):
      out = hT^T @ Wo (16 accumulating matmuls over H), scaled per-token by
      the routing weight (DVE per-partition scalar), DMA'd out.
Weights are loaded as 16 tiles of (128, 512) per tensor so the first matmuls
only wait on the first 256KB of DMA instead of the full 12.6MB.
"""

import numpy as np

D = 512
H = 2048
E = 8
P = 128
MT = 512  # max token chunk (fp32 moving-operand limit)
ND = D // P  # 4 k-tiles over D
NH = H // P  # 16 k-tiles over H
NJ = 4  # column chunks per weight d-tile (H / 512)

_CACHE: dict = {}


def _build_program(NT: int):
    from contextlib import ExitStack

    import concourse.bacc as bacc
    import concourse.tile as tile
    import concourse.mybir as mybir

    assert NT % P == 0
    f32 = mybir.dt.float32
    DT = mybir.dt.float32r

    nc = bacc.Bacc("TRN2", target_bir_lowering=False, debug=False)

    xp = nc.dram_tensor("xp", [D * NT], DT, kind="ExternalInput")
    wg = nc.dram_tensor("wg", [D, H], DT, kind="ExternalInput")
    wv = nc.dram_tensor("wv", [D, H], DT, kind="ExternalInput")
    wo = nc.dram_tensor("wo", [H, D], DT, kind="ExternalInput")
    wc = nc.dram_tensor("wc", [P, NT // P], f32, kind="ExternalInput")
    out = nc.dram_tensor("out", [NT, D], f32, kind="ExternalOutput")

    chunks = [MT] * (NT // MT)
    if NT % MT:
        chunks.append(NT % MT)
    gelu = mybir.ActivationFunctionType.Gelu

    with tile.TileContext(nc) as tc, ExitStack() as ctx:
        wpool = ctx.enter_context(tc.tile_pool(name="w", bufs=1))
        xpool = ctx.enter_context(tc.tile_pool(name="x", bufs=3))
        hpool = ctx.enter_context(tc.tile_pool(name="h", bufs=1))
        gpool = ctx.enter_context(tc.tile_pool(name="g", bufs=3))
        opool = ctx.enter_context(tc.tile_pool(name="o", bufs=4))
        psg = ctx.enter_context(tc.tile_pool(name="psg", bufs=2, space="PSUM"))
        psv = ctx.enter_context(tc.tile_pool(name="psv", bufs=2, space="PSUM"))
        pso = ctx.enter_context(tc.tile_pool(name="pso", bufs=2, space="PSUM"))

        # DMA layout notes: HWDGE packet size = free-dim bytes per partition
        # row, and per-queue bandwidth is packet-rate-bound (~150GB/s at 2KB
        # packets, ~300GB/s at 8KB). So weights load as whole-d (128, 2048)
        # tiles (8KB rows), demand-ordered across the two HWDGE queues (SP:
        # tokens+Wv, ACT: Wg+Wo) with the Wo tail on SWDGE.
        wg_t = [wpool.tile([P, H], DT, tag=f"wg{d}", name=f"wg{d}") for d in range(ND)]
        wv_t = [wpool.tile([P, H], DT, tag=f"wv{d}", name=f"wv{d}") for d in range(ND)]
        wo_t = [wpool.tile([P, D], DT, tag=f"wo{h}", name=f"wo{h}") for h in range(NH)]
        wc_t = wpool.tile([P, NT // P], f32, tag="wc")

        import concourse.bass as bass_mod

        def xp_ap(off_elems, mt):
            return bass_mod.AP(tensor=xp, offset=off_elems, ap=[[mt, P], [1, mt]])

        mt0 = chunks[0]
        xq0 = [xpool.tile([P, mt0], DT, tag=f"xq{d}", name=f"xq{d}") for d in range(ND)]
        for d in range(ND):
            nc.sync.dma_start(out=xq0[d][:], in_=xp_ap(d * P * mt0, mt0))
        nc.sync.dma_start(out=wc_t[:], in_=wc[:])
        for d in range(ND):
            nc.scalar.dma_start(out=wg_t[d][:], in_=wg[d * P : (d + 1) * P, :])
            nc.sync.dma_start(out=wv_t[d][:], in_=wv[d * P : (d + 1) * P, :])
        for h in range(NH - 8):
            nc.scalar.dma_start(out=wo_t[h][:], in_=wo[h * P : (h + 1) * P, :])
        for h in range(NH - 8, NH):
            nc.gpsimd.dma_start(out=wo_t[h][:], in_=wo[h * P : (h + 1) * P, :])

        tok0 = 0
        xp_off = 0
        for ci, mt in enumerate(chunks):
            if ci == 0:
                xq = xq0
            else:
                xq = [xpool.tile([P, mt], DT, tag=f"xq{d}", name=f"xq{d}") for d in range(ND)]
                for d in range(ND):
                    nc.sync.dma_start(out=xq[d][:], in_=xp_ap(xp_off + d * P * mt, mt))
            hT = hpool.tile([P, NH, mt], DT, tag="hT")
            for h in range(NH):
                hs = slice(h * P, (h + 1) * P)
                pg = psg.tile([P, mt], f32, tag="pg")
                pv = psv.tile([P, mt], f32, tag="pv")
                for d in range(ND):
                    nc.tensor.matmul(
                        out=pg[:], lhsT=wg_t[d][:, hs], rhs=xq[d][:],
                        start=(d == 0), stop=(d == ND - 1),
                    )
                for d in range(ND):
                    nc.tensor.matmul(
                        out=pv[:], lhsT=wv_t[d][:, hs], rhs=xq[d][:],
                        start=(d == 0), stop=(d == ND - 1),
                    )
                ga = gpool.tile([P, mt], DT, tag="ga")
                nc.scalar.activation(ga[:], pg[:], gelu)
                nc.vector.tensor_mul(hT[:, h, :], ga[:], pv[:])
            for t in range(mt // P):
                po = pso.tile([P, D], f32, tag="po")
                for h in range(NH):
                    nc.tensor.matmul(
                        out=po[:], lhsT=hT[:, h, t * P : (t + 1) * P],
                        rhs=wo_t[h][:], start=(h == 0), stop=(h == NH - 1),
                    )
                ob = opool.tile([P, D], f32, tag="ob")
                j = tok0 // P + t
                nc.vector.tensor_scalar_mul(ob[:], po[:], wc_t[:, j : j + 1])
                nc.scalar.dma_start(out=out[j * P : (j + 1) * P, :], in_=ob[:])
            tok0 += mt
            xp_off += ND * P * mt

    nc.compile()
    return nc


def kernel(tokens, dispatch_weights, combine_weights, Wg, Wv, Wo, scales):
    from concourse.bass_utils import run_bass_kernel_spmd

    B, N, d_model = tokens.shape
    M = B * N
    x = np.ascontiguousarray(tokens.reshape(M, d_model), dtype=np.float32)
    disp = np.asarray(dispatch_weights).reshape(M, E)
    comb = np.asarray(combine_weights).reshape(M, E)
    w_all = np.where(disp > 0, comb, 0.0).astype(np.float32) * np.asarray(
        scales, np.float32
    )[None, :]

    idx = [np.nonzero(w_all[:, e])[0] for e in range(E)]
    n_max = max(1, max(len(i) for i in idx))
    # pad to a multiple of 256: float32r matmuls need a moving dim >= 256 to
    # run at full rate, so a 128-wide tail chunk would cost as much as 512
    NT = ((n_max + 255) // 256) * 256

    if NT not in _CACHE:
        _CACHE[NT] = _build_program(NT)
    nc = _CACHE[NT]

    chunks = [512] * (NT // 512)
    if NT % 512:
        chunks.append(NT % 512)
    in_maps = []
    for e in range(E):
        ie = idx[e]
        ne = len(ie)
        xT_e = np.zeros((D, NT), np.float32)
        xT_e[:, :ne] = x[ie].T
        # pack chunk-major, d-major: (chunk, d, 128, mt) blocks, flattened
        xp_e = np.empty(D * NT, np.float32)
        off = 0
        tok = 0
        for mt in chunks:
            blk = xT_e[:, tok : tok + mt].reshape(ND, P, mt)
            xp_e[off : off + ND * P * mt] = blk.reshape(-1)
            off += ND * P * mt
            tok += mt
        wc_e = np.zeros((NT // P, P), np.float32)
        wc_e.reshape(-1)[:ne] = w_all[ie, e]
        in_maps.append(
            {
                "xp": xp_e,
                "wg": np.ascontiguousarray(Wg[e], np.float32),
                "wv": np.ascontiguousarray(Wv[e], np.float32),
                "wo": np.ascontiguousarray(Wo[e], np.float32),
                "wc": np.ascontiguousarray(wc_e.T),
            }
        )

    res = run_bass_kernel_spmd(nc, in_maps, list(range(E)))

    out = np.zeros((M, d_model), np.float32)
    for e in range(E):
        ne = len(idx[e])
        out[idx[e]] += res.results[e]["out"][:ne]
    return out.reshape(B, N, d_model)



# revision 4
# speedup vs baseline: 1.1799x; 1.1799x over previous
"""Trainium2 Bass kernel for the AllGroupsExpertRunner MoE problem.

Math (dense-masked reference):
    x = tokens.reshape(M, D)                                # M = B*N = 8192
    out = sum_e w[:, e] * (gelu(x @ Wg[e]) * (x @ Wv[e])) @ Wo[e] * scales[e]
    where w = where(dispatch > 0, combine, 0)

Sharding: expert-parallel - core e runs expert e on the tokens routed to it
(gathered on host, padded to a common length NT), and the host scatter-adds
the 8 partial outputs.

v2 changes vs the fp32r baseline (254us):
- All matmul operands are bf16 (weights, x, hT). PSUM accumulation stays
  fp32 and the PE multiplies at >=bf16 precision, so end-to-end rel err is
  ~4e-3 (measured vs fp64 on host), well inside the 2e-2 gate. bf16 halves
  all DMA traffic and enables fast weight load (fp32r runs LDWEIGHTS in
  fp32_mode=HIGH, which disables FWL).
- Weights are laid out per-H-block: wgv[h] = [128, 1024] holding the four
  128x128 d-blocks of Wg[:, h-block] then of Wv[:, h-block]. The first
  matmul now only waits on x-chunk0 (0.5MB) + wgv[0] (0.25MB) instead of
  all of Wg+Wv (8.4MB): PE start moves from 13.7us to ~2us and the 26us of
  early DMA stalls (which also held the HAM clock at 4/8 until 52us)
  disappear.
- DMA demand order: SP ring carries x chunks + wgv tiles, Act ring carries
  wc + wo tiles + output stores.
"""

import numpy as np
import ml_dtypes

D = 512
H = 2048
E = 8
P = 128
MT = 512  # token chunk
ND = D // P  # 4 k-tiles over D
NH = H // P  # 16 k-tiles over H

BF16 = ml_dtypes.bfloat16

_CACHE: dict = {}


def _chunks_for(NT):
    chunks = [MT] * (NT // MT)
    if NT % MT:
        chunks.append(NT % MT)
    return chunks


def _build_program(NT: int, act_name: str = "Gelu"):
    from contextlib import ExitStack

    import concourse.bacc as bacc
    import concourse.tile as tile
    import concourse.mybir as mybir

    assert NT % 256 == 0
    f32 = mybir.dt.float32
    BF = mybir.dt.bfloat16

    nc = bacc.Bacc("TRN2", target_bir_lowering=False, debug=False)

    xp = nc.dram_tensor("xp", [P, ND * NT], BF, kind="ExternalInput")
    wgv = nc.dram_tensor("wgv", [NH, P, 2 * D], BF, kind="ExternalInput")
    wo = nc.dram_tensor("wo", [NH, P, D], BF, kind="ExternalInput")
    wc = nc.dram_tensor("wc", [P, NT // P], f32, kind="ExternalInput")
    out = nc.dram_tensor("out", [NT, D], BF, kind="ExternalOutput")

    chunks = _chunks_for(NT)
    gelu = getattr(mybir.ActivationFunctionType, act_name)

    with tile.TileContext(nc) as tc, ExitStack() as ctx:
        wpool = ctx.enter_context(tc.tile_pool(name="w", bufs=1))
        xpool = ctx.enter_context(tc.tile_pool(name="x", bufs=3))
        hpool = ctx.enter_context(tc.tile_pool(name="h", bufs=2))
        gpool = ctx.enter_context(tc.tile_pool(name="g", bufs=3))
        opool = ctx.enter_context(tc.tile_pool(name="o", bufs=4))
        psg = ctx.enter_context(tc.tile_pool(name="psg", bufs=2, space="PSUM"))
        psv = ctx.enter_context(tc.tile_pool(name="psv", bufs=2, space="PSUM"))
        pso = ctx.enter_context(tc.tile_pool(name="pso", bufs=2, space="PSUM"))

        wgv_t = [wpool.tile([P, 2 * D], BF, tag=f"wgv{h}", name=f"wgv{h}") for h in range(NH)]
        wo_t = [wpool.tile([P, D], BF, tag=f"wo{h}", name=f"wo{h}") for h in range(NH)]
        wc_t = wpool.tile([P, NT // P], f32, tag="wc")

        # SP ring: x chunk 0, then all wgv tiles (demand order), later x
        # chunks are enqueued inside the loop. Act ring: wc, wo tiles, then
        # output stores from the loop.
        mt0 = chunks[0]
        xq0 = xpool.tile([P, ND * mt0], BF, tag="xq", name="xq0")
        nc.sync.dma_start(out=xq0[:], in_=xp[:, 0 : ND * mt0])
        for h in range(NH):
            nc.sync.dma_start(out=wgv_t[h][:], in_=wgv[h])
        nc.scalar.dma_start(out=wc_t[:], in_=wc[:])
        for h in range(NH):
            nc.scalar.dma_start(out=wo_t[h][:], in_=wo[h])

        tok0 = 0
        for ci, mt in enumerate(chunks):
            if ci == 0:
                xq = xq0
            else:
                xq = xpool.tile([P, ND * mt], BF, tag="xq", name=f"xq{ci}")
                nc.sync.dma_start(out=xq[:], in_=xp[:, ND * tok0 : ND * (tok0 + mt)])
            hT = hpool.tile([P, NH, mt], BF, tag="hT")
            for h in range(NH):
                pg = psg.tile([P, mt], f32, tag="pg")
                pv = psv.tile([P, mt], f32, tag="pv")
                for d in range(ND):
                    nc.tensor.matmul(
                        out=pg[:], lhsT=wgv_t[h][:, d * P : (d + 1) * P],
                        rhs=xq[:, d * mt : (d + 1) * mt],
                        start=(d == 0), stop=(d == ND - 1),
                    )
                for d in range(ND):
                    nc.tensor.matmul(
                        out=pv[:], lhsT=wgv_t[h][:, D + d * P : D + (d + 1) * P],
                        rhs=xq[:, d * mt : (d + 1) * mt],
                        start=(d == 0), stop=(d == ND - 1),
                    )
                ga = gpool.tile([P, mt], BF, tag="ga")
                nc.scalar.activation(ga[:], pg[:], gelu)
                nc.vector.tensor_mul(hT[:, h, :], ga[:], pv[:])
            for t in range(mt // P):
                po = pso.tile([P, D], f32, tag="po")
                for h in range(NH):
                    nc.tensor.matmul(
                        out=po[:], lhsT=hT[:, h, t * P : (t + 1) * P],
                        rhs=wo_t[h][:], start=(h == 0), stop=(h == NH - 1),
                    )
                ob = opool.tile([P, D], BF, tag="ob")
                j = tok0 // P + t
                nc.vector.tensor_scalar_mul(ob[:], po[:], wc_t[:, j : j + 1])
                nc.scalar.dma_start(out=out[j * P : (j + 1) * P, :], in_=ob[:])
            tok0 += mt

    nc.compile()
    return nc


def _pack_core_inputs(x_rows, Wg_e, Wv_e, Wo_e, w_tok, NT):
    """Build the in_map for one core.

    x_rows: (ne, D) float32 tokens routed to this expert
    w_tok:  (ne,) combined routing weight (includes expert scale)
    """
    ne = x_rows.shape[0]
    chunks = _chunks_for(NT)

    xT = np.zeros((D, NT), np.float32)
    xT[:, :ne] = x_rows.T
    xp = np.empty((P, ND * NT), BF16)
    tok0 = 0
    for mt in chunks:
        blk = xT[:, tok0 : tok0 + mt].reshape(ND, P, mt).transpose(1, 0, 2)
        xp[:, ND * tok0 : ND * (tok0 + mt)] = blk.reshape(P, ND * mt).astype(BF16)
        tok0 += mt

    # wgv[h] = [P, 2D]: tile[p, d*128+j] = Wg[d*128+p, h*128+j]; cols D..2D same for Wv
    wg4 = Wg_e.reshape(ND, P, H).transpose(1, 0, 2)  # (P, ND, H)
    wv4 = Wv_e.reshape(ND, P, H).transpose(1, 0, 2)
    wgv = np.empty((NH, P, 2 * D), BF16)
    for h in range(NH):
        hs = slice(h * P, (h + 1) * P)
        wgv[h, :, :D] = wg4[:, :, hs].reshape(P, D).astype(BF16)
        wgv[h, :, D:] = wv4[:, :, hs].reshape(P, D).astype(BF16)

    wo = np.ascontiguousarray(Wo_e.reshape(NH, P, D)).astype(BF16)

    wc = np.zeros((NT // P, P), np.float32)
    wc.reshape(-1)[:ne] = w_tok
    return {
        "xp": xp,
        "wgv": wgv,
        "wo": wo,
        "wc": np.ascontiguousarray(wc.T),
    }


def kernel(tokens, dispatch_weights, combine_weights, Wg, Wv, Wo, scales):
    from concourse.bass_utils import run_bass_kernel_spmd

    B, N, d_model = tokens.shape
    M = B * N
    x = np.ascontiguousarray(tokens.reshape(M, d_model), dtype=np.float32)
    disp = np.asarray(dispatch_weights).reshape(M, E)
    comb = np.asarray(combine_weights).reshape(M, E)
    w_all = np.where(disp > 0, comb, 0.0).astype(np.float32) * np.asarray(
        scales, np.float32
    )[None, :]

    idx = [np.nonzero(w_all[:, e])[0] for e in range(E)]
    n_max = max(1, max(len(i) for i in idx))
    NT = ((n_max + 255) // 256) * 256

    if NT not in _CACHE:
        _CACHE[NT] = _build_program(NT)
    nc = _CACHE[NT]

    in_maps = [
        _pack_core_inputs(
            x[idx[e]],
            np.asarray(Wg[e], np.float32),
            np.asarray(Wv[e], np.float32),
            np.asarray(Wo[e], np.float32),
            w_all[idx[e], e],
            NT,
        )
        for e in range(E)
    ]

    res = run_bass_kernel_spmd(nc, in_maps, list(range(E)))

    out = np.zeros((M, d_model), np.float32)
    for e in range(E):
        ne = len(idx[e])
        out[idx[e]] += res.results[e]["out"][:ne].astype(np.float32)
    return out.reshape(B, N, d_model)


# revision 8
# speedup vs baseline: 1.1848x; 1.0041x over previous
"""Trainium2 Bass kernel for the AllGroupsExpertRunner MoE problem.

Math (dense-masked reference):
    x = tokens.reshape(M, D)                                # M = B*N = 8192
    out = sum_e w[:, e] * (gelu(x @ Wg[e]) * (x @ Wv[e])) @ Wo[e] * scales[e]
    where w = where(dispatch > 0, combine, 0)

Sharding: expert-parallel - core e runs expert e on the tokens routed to it
(gathered on host, padded to a common length NT), and the host scatter-adds
the 8 partial outputs.

v2 changes vs the fp32r baseline (254us):
- All matmul operands are bf16 (weights, x, hT). PSUM accumulation stays
  fp32 and the PE multiplies at >=bf16 precision, so end-to-end rel err is
  ~4e-3 (measured vs fp64 on host), well inside the 2e-2 gate. bf16 halves
  all DMA traffic and enables fast weight load (fp32r runs LDWEIGHTS in
  fp32_mode=HIGH, which disables FWL).
- Weights are laid out per-H-block: wgv[h] = [128, 1024] holding the four
  128x128 d-blocks of Wg[:, h-block] then of Wv[:, h-block]. The first
  matmul now only waits on x-chunk0 (0.5MB) + wgv[0] (0.25MB) instead of
  all of Wg+Wv (8.4MB): PE start moves from 13.7us to ~2us and the 26us of
  early DMA stalls (which also held the HAM clock at 4/8 until 52us)
  disappear.
- DMA demand order: SP ring carries x chunks + wgv tiles, Act ring carries
  wc + wo tiles + output stores.
"""

import numpy as np
import ml_dtypes

D = 512
H = 2048
E = 8
P = 128
MT = 512  # token chunk
ND = D // P  # 4 k-tiles over D
NH = H // P  # 16 k-tiles over H

BF16 = ml_dtypes.bfloat16

_CACHE: dict = {}


def _chunks_for(NT):
    chunks = [MT] * (NT // MT)
    if NT % MT:
        chunks.append(NT % MT)
    return chunks


def _build_program(NT: int, act_name: str = "Gelu"):
    from contextlib import ExitStack

    import concourse.bacc as bacc
    import concourse.tile as tile
    import concourse.mybir as mybir

    assert NT % 256 == 0
    f32 = mybir.dt.float32
    BF = mybir.dt.bfloat16

    nc = bacc.Bacc("TRN2", target_bir_lowering=False, debug=False)

    xp = nc.dram_tensor("xp", [P, ND * NT], BF, kind="ExternalInput")
    # weight pairs: wgv[q] holds H-blocks h=2q (cols 0:1024) and 2q+1
    # (cols 1024:2048); within a block, [Wg d-major 512 | Wv d-major 512].
    # 4KB rows for good DMA packet rate; pairs alternate between the two
    # HWDGE rings so weight arrival keeps ahead of stage-A demand.
    wgv = nc.dram_tensor("wgv", [NH // 2, P, 4 * D], BF, kind="ExternalInput")
    # wo quads: wo4[q] cols (h%4)*512 hold Wo[h*128:(h+1)*128, :]
    wo4 = nc.dram_tensor("wo4", [NH // 4, P, 4 * D], BF, kind="ExternalInput")
    wc = nc.dram_tensor("wc", [P, NT // P], f32, kind="ExternalInput")
    out = nc.dram_tensor("out", [NT, D], BF, kind="ExternalOutput")

    chunks = _chunks_for(NT)
    gelu = getattr(mybir.ActivationFunctionType, act_name)

    with tile.TileContext(nc) as tc, ExitStack() as ctx:
        wpool = ctx.enter_context(tc.tile_pool(name="w", bufs=1))
        xpool = ctx.enter_context(tc.tile_pool(name="x", bufs=3))
        hpool = ctx.enter_context(tc.tile_pool(name="h", bufs=2))
        gpool = ctx.enter_context(tc.tile_pool(name="g", bufs=3))
        opool = ctx.enter_context(tc.tile_pool(name="o", bufs=4))
        psg = ctx.enter_context(tc.tile_pool(name="psg", bufs=2, space="PSUM"))
        psv = ctx.enter_context(tc.tile_pool(name="psv", bufs=2, space="PSUM"))
        pso = ctx.enter_context(tc.tile_pool(name="pso", bufs=2, space="PSUM"))

        wgv_t = [wpool.tile([P, 4 * D], BF, tag=f"wgv{q}", name=f"wgv{q}") for q in range(NH // 2)]
        wo_t = [wpool.tile([P, 4 * D], BF, tag=f"wo{q}", name=f"wo{q}") for q in range(NH // 4)]
        wc_t = wpool.tile([P, NT // P], f32, tag="wc")

        # SP ring: x0 d-block 0, first weight pair, rest of x0, even weight
        # pairs, then later x chunks in-loop. Act ring: wc, odd weight
        # pairs, then output stores from the loop. SWDGE: wo quads.
        mt0 = chunks[0]
        xq0 = xpool.tile([P, ND * mt0], BF, tag="xq", name="xq0")
        nc.sync.dma_start(out=xq0[:, :mt0], in_=xp[:, 0:mt0])
        nc.sync.dma_start(out=wgv_t[0][:], in_=wgv[0])
        nc.sync.dma_start(out=xq0[:, mt0:], in_=xp[:, mt0 : ND * mt0])
        for q in range(2, NH // 2, 2):
            nc.sync.dma_start(out=wgv_t[q][:], in_=wgv[q])
        nc.scalar.dma_start(out=wc_t[:], in_=wc[:])
        for q in range(1, NH // 2, 2):
            nc.scalar.dma_start(out=wgv_t[q][:], in_=wgv[q])
        for q in range(NH // 4):
            nc.gpsimd.dma_start(out=wo_t[q][:], in_=wo4[q])

        tok0 = 0
        for ci, mt in enumerate(chunks):
            if ci == 0:
                xq = xq0
            else:
                xq = xpool.tile([P, ND * mt], BF, tag="xq", name=f"xq{ci}")
                nc.sync.dma_start(out=xq[:], in_=xp[:, ND * tok0 : ND * (tok0 + mt)])
            hT = hpool.tile([P, NH, mt], BF, tag="hT")
            for h in range(NH):
                base = (h % 2) * 2 * D
                pg = psg.tile([P, mt], f32, tag="pg")
                pv = psv.tile([P, mt], f32, tag="pv")
                for d in range(ND):
                    nc.tensor.matmul(
                        out=pg[:],
                        lhsT=wgv_t[h // 2][:, base + d * P : base + (d + 1) * P],
                        rhs=xq[:, d * mt : (d + 1) * mt],
                        start=(d == 0), stop=(d == ND - 1),
                    )
                for d in range(ND):
                    nc.tensor.matmul(
                        out=pv[:],
                        lhsT=wgv_t[h // 2][:, base + D + d * P : base + D + (d + 1) * P],
                        rhs=xq[:, d * mt : (d + 1) * mt],
                        start=(d == 0), stop=(d == ND - 1),
                    )
                ga = gpool.tile([P, mt], BF, tag="ga")
                nc.scalar.activation(ga[:], pg[:], gelu)
                nc.vector.tensor_mul(hT[:, h, :], ga[:], pv[:])
            for t in range(mt // P):
                po = pso.tile([P, D], f32, tag="po")
                for h in range(NH):
                    nc.tensor.matmul(
                        out=po[:], lhsT=hT[:, h, t * P : (t + 1) * P],
                        rhs=wo_t[h // 4][:, (h % 4) * D : (h % 4 + 1) * D],
                        start=(h == 0), stop=(h == NH - 1),
                    )
                ob = opool.tile([P, D], BF, tag="ob")
                j = tok0 // P + t
                nc.vector.tensor_scalar_mul(ob[:], po[:], wc_t[:, j : j + 1])
                nc.scalar.dma_start(out=out[j * P : (j + 1) * P, :], in_=ob[:])
            tok0 += mt

    nc.compile()
    return nc


def _pack_core_inputs(x_rows, Wg_e, Wv_e, Wo_e, w_tok, NT):
    """Build the in_map for one core.

    x_rows: (ne, D) float32 tokens routed to this expert
    w_tok:  (ne,) combined routing weight (includes expert scale)
    """
    ne = x_rows.shape[0]
    chunks = _chunks_for(NT)

    xT = np.zeros((D, NT), np.float32)
    xT[:, :ne] = x_rows.T
    xp = np.empty((P, ND * NT), BF16)
    tok0 = 0
    for mt in chunks:
        blk = xT[:, tok0 : tok0 + mt].reshape(ND, P, mt).transpose(1, 0, 2)
        xp[:, ND * tok0 : ND * (tok0 + mt)] = blk.reshape(P, ND * mt).astype(BF16)
        tok0 += mt

    # wgv[q] = [P, 4D] holding H-blocks h=2q, 2q+1; within each block
    # [Wg d-major 512 | Wv d-major 512]: block[p, d*128+j] = Wg[d*128+p, h*128+j]
    wg4 = Wg_e.reshape(ND, P, H).transpose(1, 0, 2)  # (P, ND, H)
    wv4 = Wv_e.reshape(ND, P, H).transpose(1, 0, 2)
    wgv = np.empty((NH // 2, P, 4 * D), BF16)
    for h in range(NH):
        hs = slice(h * P, (h + 1) * P)
        base = (h % 2) * 2 * D
        wgv[h // 2, :, base : base + D] = wg4[:, :, hs].reshape(P, D).astype(BF16)
        wgv[h // 2, :, base + D : base + 2 * D] = wv4[:, :, hs].reshape(P, D).astype(BF16)

    # wo4[q] cols (h%4)*512 hold Wo[h*128:(h+1)*128, :]
    wo4 = np.ascontiguousarray(
        Wo_e.reshape(NH // 4, 4, P, D).transpose(0, 2, 1, 3).reshape(NH // 4, P, 4 * D)
    ).astype(BF16)

    wc = np.zeros((NT // P, P), np.float32)
    wc.reshape(-1)[:ne] = w_tok
    return {
        "xp": xp,
        "wgv": wgv,
        "wo4": wo4,
        "wc": np.ascontiguousarray(wc.T),
    }


def kernel(tokens, dispatch_weights, combine_weights, Wg, Wv, Wo, scales):
    from concourse.bass_utils import run_bass_kernel_spmd

    B, N, d_model = tokens.shape
    M = B * N
    x = np.ascontiguousarray(tokens.reshape(M, d_model), dtype=np.float32)
    disp = np.asarray(dispatch_weights).reshape(M, E)
    comb = np.asarray(combine_weights).reshape(M, E)
    w_all = np.where(disp > 0, comb, 0.0).astype(np.float32) * np.asarray(
        scales, np.float32
    )[None, :]

    idx = [np.nonzero(w_all[:, e])[0] for e in range(E)]
    n_max = max(1, max(len(i) for i in idx))
    NT = ((n_max + 255) // 256) * 256

    if NT not in _CACHE:
        _CACHE[NT] = _build_program(NT)
    nc = _CACHE[NT]

    in_maps = [
        _pack_core_inputs(
            x[idx[e]],
            np.asarray(Wg[e], np.float32),
            np.asarray(Wv[e], np.float32),
            np.asarray(Wo[e], np.float32),
            w_all[idx[e], e],
            NT,
        )
        for e in range(E)
    ]

    res = run_bass_kernel_spmd(nc, in_maps, list(range(E)))

    out = np.zeros((M, d_model), np.float32)
    for e in range(E):
        ne = len(idx[e])
        out[idx[e]] += res.results[e]["out"][:ne].astype(np.float32)
    return out.reshape(B, N, d_model)
